# revision 3
# baseline (speedup 1.0000x reference)
"""CIoU kernel v3 (T=64): as kernel v2 but with halved instruction count and
SBUF aliasing so the larger chunk fits:
  - 9x9 padded C (wrap row/col appended once, core at [0:8,0:8]);
  - 9-slot SA/SB pads; rAc aliases cpad, qA/qB aliases w1p (slice views);
  - fp16 bridge tiles carved out of dead sap/sbp regions via bitcast;
  - shf kept fp16 (+-64 exact), mixed-dtype Pool subtract (probed legal).
Algorithm identical to kernel v2 (validated: rel_err 2.3e-05).
"""
import sys

sys.path.insert(0, "/opt/trn_rl_repo")

import numpy as np
import concourse.bass as bass
import concourse.bacc as bacc
import concourse.tile as tile
from concourse import mybir
from concourse.bass_utils import run_bass_kernel_spmd

AOT = mybir.AluOpType
ACT = mybir.ActivationFunctionType
F32 = mybir.dt.float32
F16 = mybir.dt.float16

B = 262144
NCORES = 8
NI = B // NCORES
P = 128
EPS = 1e-12
BIG = 1e20
SHIFT = 64.0
TINY = 1e-30


ASSIGN = {}


def build_program(T=64, assign=None):
    global ASSIGN
    ASSIGN = dict(assign or {})
    CH = P * T
    NCH = NI // CH
    nc = bacc.Bacc("TRN2", target_bir_lowering=False, debug=False, num_devices=NCORES)
    ab_d = nc.dram_tensor("ab", [NI, 32], F32, kind="ExternalInput")
    out_d = nc.dram_tensor("ciou", [NI], F32, kind="ExternalOutput")

    with tile.TileContext(nc) as tc:
        with tc.tile_pool(name="pool", bufs=1) as pool, \
             tc.tile_pool(name="spool", bufs=1) as spool:
            tb = spool.tile([P, 1], F32, tag="tinyb", name="tinyb")
            nc.gpsimd.memset(tb, TINY)
            for ch in range(NCH):
                _chunk(nc, pool, spool, ab_d, out_d, ch, T, tb)
    nc.compile()
    return nc


def _chunk(nc, pool, spool, ab_d, out_d, ch, T, tb):
    CH = P * T
    v = nc.vector
    g = nc.gpsimd
    s = nc.scalar

    def E(site, default):
        return {"v": v, "g": g, "s": s}[ASSIGN.get(site, default)]

    def big(tag, d=F32, n=64):
        return pool.tile([P, n * T], d, tag=tag, name=tag)

    def small(tag, d=F32, n=8):
        return spool.tile([P, n * T], d, tag=tag, name=tag)

    def tiny(tag, d=F32):
        return spool.tile([P, T], d, tag=tag, name=tag)

    def dn(tl):
        return tl.rearrange("p (i k t) -> p i k t", i=8, k=8)

    # ---------------- load ----------------
    raw = pool.tile([P, 32 * T], F32, tag="raw", name="raw")
    ab_view = ab_d[ch * CH:(ch + 1) * CH, :].rearrange("(p t) jc -> p (t jc)", p=P)
    nc.sync.dma_start(raw, ab_view)

    rr = raw.rearrange("p (t h j c) -> p h j c t", h=2, j=8, c=2)
    ax = rr[:, 0, :, 0, :]; ay = rr[:, 0, :, 1, :]
    bx = rr[:, 1, :, 0, :]; by = rr[:, 1, :, 1, :]

    # ---------------- adjacency (smalls) ----------------
    ua1 = small("adjt1"); ua2 = small("adjt2")
    va1 = ua1.rearrange("p (s t) -> p s t", s=8)
    va2 = ua2.rearrange("p (s t) -> p s t", s=8)
    v.tensor_tensor(va1[:, 0:7, :], ax[:, 0:7, :], ay[:, 1:8, :], AOT.mult)
    v.tensor_tensor(va2[:, 0:7, :], ay[:, 0:7, :], ax[:, 1:8, :], AOT.mult)
    v.tensor_tensor(va1[:, 7, :], ax[:, 7, :], ay[:, 0, :], AOT.mult)
    v.tensor_tensor(va2[:, 7, :], ay[:, 7, :], ax[:, 0, :], AOT.mult)
    adjAB = small("adjab", n=16)
    adjAB_v = adjAB.rearrange("p (s t) -> p s t", s=16)
    adjA_c = adjAB_v[:, 0:8, :]
    v.tensor_tensor(adjA_c, va1, va2, AOT.subtract)

    ub1 = small("adjt1"); ub2 = small("adjt2")
    vb1 = ub1.rearrange("p (s t) -> p s t", s=8)
    vb2 = ub2.rearrange("p (s t) -> p s t", s=8)
    g.tensor_tensor(vb1[:, 0:7, :], bx[:, 0:7, :], by[:, 1:8, :], AOT.mult)
    g.tensor_tensor(vb2[:, 0:7, :], by[:, 0:7, :], bx[:, 1:8, :], AOT.mult)
    g.tensor_tensor(vb1[:, 7, :], bx[:, 7, :], by[:, 0, :], AOT.mult)
    g.tensor_tensor(vb2[:, 7, :], by[:, 7, :], bx[:, 0, :], AOT.mult)
    adjB_c = adjAB_v[:, 8:16, :]
    g.tensor_tensor(adjB_c, vb1, vb2, AOT.subtract)

    adjA_ik = adjA_c.unsqueeze(2).broadcast_to((P, 8, 8, T))
    adjB_ik = adjB_c.unsqueeze(1).broadcast_to((P, 8, 8, T))

    # ---------------- 9x9 padded C (core [0:8,0:8], wrap row/col 8) --------
    t1 = big("t1")
    t2 = big("t2")
    ax_b = ax.unsqueeze(2).broadcast_to((P, 8, 8, T))
    ay_b = ay.unsqueeze(2).broadcast_to((P, 8, 8, T))
    bx_b = bx.unsqueeze(1).broadcast_to((P, 8, 8, T))
    by_b = by.unsqueeze(1).broadcast_to((P, 8, 8, T))
    v.tensor_tensor(dn(t1), ax_b, by_b, AOT.mult)
    g.tensor_tensor(dn(t2), ay_b, bx_b, AOT.mult)

    cpad = big("cpad", n=81)
    cp = cpad.rearrange("p (si sk t) -> p si sk t", si=9, sk=9)
    C0 = cp[:, 0:8, 0:8, :]
    v.tensor_tensor(C0, dn(t1), dn(t2), AOT.subtract)
    s.copy(cp[:, 0:8, 8, :], cp[:, 0:8, 0, :])      # col 8 <- col 0
    s.copy(cp[:, 8, :, :], cp[:, 0, :, :])          # row 8 <- row 0 (incl corner)

    # fp16 C for the bridge sum
    cf = big("cf", d=F16)
    s.copy(dn(cf), C0)

    # ---------------- SA / SB / D ----------------
    # w1[i, q] = C[i+1, q] - C[i, q], q = 0..8
    w1p = big("w1p", n=72)
    w1v = w1p.rearrange("p (i q t) -> p i q t", i=8, q=9)
    v.tensor_tensor(w1v, cp[:, 1:9, :, :], cp[:, 0:8, :, :], AOT.subtract)

    sap = big("sap", n=72)          # 9 i-slots: SAm=0:8, SA=1:9, slot0<-slot8
    sav = sap.rearrange("p (si k t) -> p si k t", si=9, k=8)
    SA = sav[:, 1:9, :, :]
    E("sa", "g").tensor_tensor(SA, w1v[:, :, 0:8, :], adjA_ik, AOT.add)
    s.copy(sav[:, 0, :, :], sav[:, 8, :, :])

    w2 = big("w2")
    E("w2", "g").tensor_tensor(dn(w2), cp[:, 0:8, 0:8, :], cp[:, 0:8, 1:9, :], AOT.subtract)
    sbp = big("sbp", n=72)          # 9 k-slots: SBm=0:8, SB=1:9, slot0<-slot8
    sbv = sbp.rearrange("p (i sk t) -> p i sk t", i=8, sk=9)
    SB = sbv[:, :, 1:9, :]
    E("sb", "v").tensor_tensor(SB, dn(w2), adjB_ik, AOT.add)
    s.copy(sbv[:, :, 0, :], sbv[:, :, 8, :])

    D = big("t1")                   # reuse t1 slot
    E("d", "g").tensor_tensor(dn(D), w1v[:, :, 1:9, :], w1v[:, :, 0:8, :], AOT.subtract)

    # ---------------- D sign chain ----------------
    sgnf = big("sgn", d=F16)
    s.activation(sgnf, D, ACT.Sign, bias=tb)
    Dsafe = big("t2")               # reuse t2 slot
    v.scalar_tensor_tensor(Dsafe, sgnf, EPS, D, op0=AOT.mult, op1=AOT.add)
    R = big("r")
    v.reciprocal(R, Dsafe)
    Rv = R.rearrange("p (i k t) -> p i k t", i=8, k=8)
    shf = big("shf", d=F16)
    g.tensor_scalar(shf, sgnf, SHIFT, None, AOT.mult)

    # ---------------- clip chains (f32) ----------------
    ratioA = big("t1")              # reuse
    v.tensor_tensor(dn(ratioA), SB, Rv, AOT.mult)
    rAc = cpad[:, 0:64 * T]         # alias: C table is dead by now
    g.tensor_scalar(rAc, ratioA, 4.0, -4.0, AOT.min, AOT.max)
    qA = w1p[:, 0:64 * T]           # alias: w1 dead after SA/D
    g.tensor_tensor(qA, rAc, shf, AOT.subtract)
    loA = small("loa")
    v.tensor_reduce(loA.rearrange("p (i t) -> p i t", i=8),
                    qA.rearrange("p (i k t) -> p i t k", i=8, k=8),
                    axis=mybir.AxisListType.X, op=AOT.max)
    hiA = small("hia")
    v.tensor_reduce(hiA.rearrange("p (i t) -> p i t", i=8),
                    qA.rearrange("p (i k t) -> p i t k", i=8, k=8),
                    axis=mybir.AxisListType.X, op=AOT.min)

    ratioB = big("t2")              # reuse
    E("rb", "v").tensor_tensor(dn(ratioB), SA, Rv, AOT.mult)
    rBc = cpad[:, 0:64 * T]         # alias (rAc consumed by qA)
    g.tensor_scalar(rBc, ratioB, 4.0, -4.0, AOT.min, AOT.max)
    qB = big("w2")                  # w2 is dead after SB; stays on Pool
    g.tensor_tensor(qB, rBc, shf, AOT.subtract)
    loB = sgnf[:, 0:16 * T].bitcast(F32)
    v.tensor_reduce(loB.rearrange("p (k t) -> p k t", k=8),
                    qB.rearrange("p (i k t) -> p k t i", i=8, k=8),
                    axis=mybir.AxisListType.X, op=AOT.min)
    hiB = sgnf[:, 16 * T:32 * T].bitcast(F32)
    v.tensor_reduce(hiB.rearrange("p (k t) -> p k t", k=8),
                    qB.rearrange("p (i k t) -> p k t i", i=8, k=8),
                    axis=mybir.AxisListType.X, op=AOT.max)

    # widths + weighted sums (smalls); shift constants folded in
    hiA2 = R[:, 0:8 * T]
    v.tensor_scalar(hiA2, hiA, SHIFT, 1.0, AOT.add, AOT.min)
    loA2 = R[:, 8 * T:16 * T]
    v.tensor_scalar(loA2, loA, -SHIFT, 0.0, AOT.add, AOT.max)
    wA = R[:, 16 * T:24 * T]
    v.tensor_tensor(wA, hiA2, loA2, AOT.subtract)
    wAr = R[:, 24 * T:32 * T]
    v.tensor_scalar_max(wAr, wA, 0.0)
    wadjAB = ratioA[:, 16 * T:32 * T]
    wadjAB_v = wadjAB.rearrange("p (s t) -> p s t", s=16)
    v.tensor_tensor(wadjAB_v[:, 0:8, :],
                    wAr.rearrange("p (i t) -> p i t", i=8), adjA_c, AOT.mult)

    loB2 = R[:, 32 * T:40 * T]
    v.tensor_scalar(loB2, loB, SHIFT, 0.0, AOT.add, AOT.min)
    hiB2 = R[:, 40 * T:48 * T]
    v.tensor_scalar(hiB2, hiB, -SHIFT, -1.0, AOT.add, AOT.max)
    wB = R[:, 48 * T:56 * T]
    v.tensor_tensor(wB, loB2, hiB2, AOT.subtract)
    wBr = R[:, 56 * T:64 * T]
    v.tensor_scalar_max(wBr, wB, 0.0)
    v.tensor_tensor(wadjAB_v[:, 8:16, :],
                    wBr.rearrange("p (k t) -> p k t", k=8), adjB_c, AOT.mult)
    isum = tiny("isum")
    v.tensor_reduce(isum, wadjAB.rearrange("p (s t) -> p t s", s=16),
                    axis=mybir.AxisListType.X, op=AOT.add)

    # ---------------- fp16 casts of SA/SB (Act) ----------------
    saf = big("saf", d=F16, n=72)
    safv = saf.rearrange("p (si k t) -> p si k t", si=9, k=8)
    s.copy(saf, sap)
    sbf = big("sbf", d=F16, n=72)
    sbfv = sbf.rearrange("p (i sk t) -> p i sk t", i=8, sk=9)
    s.copy(sbf, sbp)

    SAf = safv[:, 1:9, :, :]
    SAm = safv[:, 0:8, :, :]
    SBf = sbfv[:, :, 1:9, :]
    SBm = sbfv[:, :, 0:8, :]

    # ---------------- hull: surviving edges ----------------
    minSA = small("minsa", d=F16)
    v.tensor_reduce(minSA.rearrange("p (i t) -> p i t", i=8),
                    SAf.rearrange("p i k t -> p i t k"),
                    axis=mybir.AxisListType.X, op=AOT.min)
    okEA = small("okea")
    v.tensor_scalar(okEA, minSA, 0.0, None, AOT.is_ge)
    eAB = ratioA[:, 0:16 * T]
    eAB_v = eAB.rearrange("p (s t) -> p s t", s=16)
    v.tensor_tensor(eAB_v[:, 0:8, :],
                    okEA.rearrange("p (i t) -> p i t", i=8), adjA_c, AOT.mult)
    minSB = small("minsa", d=F16)   # reuse
    v.tensor_reduce(minSB.rearrange("p (k t) -> p k t", k=8),
                    SBf.rearrange("p i k t -> p k t i"),
                    axis=mybir.AxisListType.X, op=AOT.min)
    okEB = small("okea")            # reuse
    v.tensor_scalar(okEB, minSB, 0.0, None, AOT.is_ge)
    v.tensor_tensor(eAB_v[:, 8:16, :],
                    okEB.rearrange("p (k t) -> p k t", k=8), adjB_c, AOT.mult)
    h1 = tiny("h1")
    v.tensor_reduce(h1, eAB.rearrange("p (s t) -> p t s", s=16),
                    axis=mybir.AxisListType.X, op=AOT.add)

    # ---------------- hull: bridges (fp16, carved from sap/sbp) ----------
    u = sap[:, 0:32 * T].bitcast(F16)
    v.tensor_tensor(dn(u), SAm, SBf, AOT.min)
    vv = sap[:, 32 * T:64 * T].bitcast(F16)
    v.tensor_tensor(dn(vv), SAf, SBm, AOT.max)
    s1 = sbp[:, 0:32 * T].bitcast(F16)
    v.scalar_tensor_tensor(s1, vv, -1.0, u, op0=AOT.mult, op1=AOT.min)
    mAB = sbp[:, 32 * T:64 * T].bitcast(F16)
    v.tensor_scalar(mAB, s1, 0.0, None, AOT.is_ge)
    u2 = sap[:, 0:32 * T].bitcast(F16)
    v.tensor_tensor(dn(u2), SAm, SBf, AOT.max)
    v2 = sap[:, 32 * T:64 * T].bitcast(F16)
    v.tensor_tensor(dn(v2), SAf, SBm, AOT.min)
    s2 = sbp[:, 0:32 * T].bitcast(F16)
    v.scalar_tensor_tensor(s2, u2, -1.0, v2, op0=AOT.mult, op1=AOT.min)
    mBA = sap[:, 0:32 * T].bitcast(F16)
    v.tensor_scalar(mBA, s2, 0.0, None, AOT.is_ge)
    mB = sap[:, 32 * T:64 * T].bitcast(F16)
    v.tensor_tensor(mB, mAB, mBA, AOT.subtract)
    cM = sbp[:, 0:32 * T].bitcast(F16)
    v.tensor_tensor(cM, mB, cf, AOT.mult)
    redBR = tiny("redbr")
    v.tensor_reduce(redBR, cM.rearrange("p (i k t) -> p t i k", i=8, k=8),
                    axis=mybir.AxisListType.XY, op=AOT.add)

    # ---------------- per-item finals ----------------
    asum = tiny("asum")
    v.tensor_reduce(asum, adjAB.rearrange("p (s t) -> p t s", s=16),
                    axis=mybir.AxisListType.X, op=AOT.add)
    inter = tiny("inter")
    s.mul(inter, isum, 0.5)
    union = tiny("union")
    v.scalar_tensor_tensor(union, asum, 0.5, inter, op0=AOT.mult, op1=AOT.subtract)
    hsum = tiny("hsum")
    v.tensor_tensor(hsum, h1, redBR, AOT.add)
    rcu = tiny("rcu")
    v.reciprocal(rcu, union)
    rch = tiny("rch")
    v.reciprocal(rch, hsum)
    iou = tiny("iou")
    v.tensor_tensor(iou, inter, rcu, AOT.mult)
    ioum1 = tiny("ioum1")
    v.tensor_scalar_add(ioum1, iou, -1.0)
    qq = tiny("qq")
    v.tensor_tensor(qq, union, rch, AOT.mult)
    ciou = tiny("ciou")
    v.scalar_tensor_tensor(ciou, qq, 2.0, ioum1, op0=AOT.mult, op1=AOT.add)
    out_view = out_d[ch * CH:(ch + 1) * CH].rearrange("(p t) -> p t", p=P)
    nc.sync.dma_start(out_view, ciou)
